# revision 11
# baseline (speedup 1.0000x reference)
"""Multi-head self-attention (B=2, S=2048, E=1024, H=16) on 8 NeuronCores.

Sharding: core c handles batch b = c // 4 and head group g = c % 4 (4 heads).
Each core computes: qkv projection for its heads, attention, and a partial
c_proj (row-slice of W_proj); the host sums the 4 partials per batch.

Device pipeline (all matmuls bf16 inputs, fp32 PSUM accumulation):
  qkT  [2*GW, S]  = (wqk slice).T-stationary @ xT          (Q^T and K^T, head-major)
  V    [S, GW]    = xT-stationary @ wv                     (natural layout, +ones col)
  sT   [kt, q]    = K^T.T @ Q^T  (per head, 64-deep contraction, row-packed pairs)
  expT            = exp(0.125 * sT)  on ACT, bf16 out      (no max-sub: |s/8| <~ 6)
  yT'  [65, q]    = V'.T @ expT  (row 64 = softmax sums via ones column)
  yn   [64, q]    = yT' * (1/sums) broadcast               (per-head normalization)
  out  [S, E]    += yn.T-stationary @ wp                   (partial; host reduces)

Schedule: warmup matmuls run on a memset tile (no DMA dep) right after the
preamble; the pre-phase consumes (wqk[e], xt[e]) pairs e-major as they land
(K^T pair0 + Q^T pair0 lo + first two V tiles); the remaining qkT work is
dripped as 2-matmul chunks through units 1-4 so the PE never starves while
the exp-bound attention windows run; c_proj of qt-1 drips into pair-1 units.
Output partials are written bf16 (host reduce in fp32).
"""

import numpy as np
import ml_dtypes

B, S, E, H, D = 2, 2048, 1024, 16, 64
HPC = 4              # heads per core
GW = HPC * D         # 256: per-core width of q/k/v blocks
VW = D + 1           # 65: v columns + ones column
NCORES = 8
ET = E // 128        # 8 contraction tiles for the projections
NQT = S // 512       # 4 query-column tiles
NKT = S // 128       # 16 key tiles
BF16 = ml_dtypes.bfloat16

_CACHE = {}


def _build():
    import concourse.bass as bass
    import concourse.mybir as mybir
    import concourse.tile as tile
    from concourse.tile import add_dep_helper
    from concourse import bacc

    f32, bf16 = mybir.dt.float32, mybir.dt.bfloat16
    Exp = mybir.ActivationFunctionType.Exp

    nc = bacc.Bacc("TRN2", target_bir_lowering=False, debug=False, num_devices=NCORES)
    xT = nc.dram_tensor("xT", [E, S], bf16, kind="ExternalInput").ap()
    wqk = nc.dram_tensor("wqk", [E, 2 * GW], bf16, kind="ExternalInput").ap()
    # wv/bqk/wp are pre-arranged host-side into single-issue 2D layouts:
    # partition p holds [t0-block | t1-block | ...] for row (t*128+p)
    wv_r = nc.dram_tensor("wv", [128, ET * GW], bf16, kind="ExternalInput").ap()
    bqk_r = nc.dram_tensor("bqk", [128, 2 * GW // 128], f32, kind="ExternalInput").ap()
    bv = nc.dram_tensor("bv", [GW], f32, kind="ExternalInput").ap()
    wp_r = nc.dram_tensor("wp", [128, 2 * E], bf16, kind="ExternalInput").ap()
    out = nc.dram_tensor("out", [S, E], bf16, kind="ExternalOutput").ap()

    xT_t = xT.rearrange("(t p) s -> t p s", p=128)
    wqk_t = wqk.rearrange("(t p) n -> t p n", p=128)
    out_t = out.rearrange("(t p) n -> t p n", p=128)

    with tile.TileContext(nc) as tc:
        with (
            tc.tile_pool(name="xp", bufs=ET) as xp,
            tc.tile_pool(name="wqkp", bufs=ET) as wqkp,
            tc.tile_pool(name="wvp", bufs=1) as wvp,
            tc.tile_pool(name="wpp", bufs=1) as wpp,
            tc.tile_pool(name="cst", bufs=1) as cst,
            tc.tile_pool(name="qkp", bufs=4) as qkp,
            tc.tile_pool(name="vp", bufs=NKT) as vp,
            tc.tile_pool(name="ep", bufs=2) as ep,
            tc.tile_pool(name="ynp", bufs=2) as ynp,
            tc.tile_pool(name="rp", bufs=4) as rp,
            tc.tile_pool(name="op", bufs=4) as op,
            tc.tile_pool(name="spool", bufs=2, space="PSUM") as spool,
            tc.tile_pool(name="ppool", bufs=4, space="PSUM") as ppool,
        ):
            # ---- constants first: warmup stationary + exp-table preload src,
            # so the PE/ACT warmups have no DMA dependency at all ----
            warm_w = cst.tile([128, 320], bf16, tag="warmw")
            nc.vector.memset(warm_w[:], 0.25)
            wexp_src = cst.tile([128, 1], f32, tag="wexpsrc")
            nc.vector.memset(wexp_src[:], 0.0)

            # ---- input DMAs: wqk[e] before xt[e] (pre-phase consumes pairs
            # e-major); small/late-needed tensors batched into single issues ----
            xt, wqk_sb = [], []
            wv_sb = wvp.tile([128, ET * GW], bf16, tag="wv")
            bqk_sb4 = cst.tile([128, 2 * GW // 128], f32, tag="bqk")
            wp_sb2 = wpp.tile([128, 2 * E], bf16, tag="wp")
            bv_bc = cst.tile([128, GW], f32, tag="bv")
            bv_b = bass.AP(tensor=bv.tensor, offset=bv.offset, ap=[[0, 128], *bv.ap])
            for e in range(ET):
                t = wqkp.tile([128, 2 * GW], bf16, tag="wqk", name=f"wqk{e}")
                nc.sync.dma_start(t[:], wqk_t[e])
                wqk_sb.append(t)
                t = xp.tile([128, S], bf16, tag="xt", name=f"xt{e}")
                nc.sync.dma_start(t[:], xT_t[e])
                xt.append(t)
                if e == 1:
                    nc.sync.dma_start(wv_sb[:], wv_r)
                    nc.sync.dma_start(bv_bc[:], bv_b)
                if e == 3:
                    nc.sync.dma_start(bqk_sb4[:], bqk_r)
            nc.sync.dma_start(wp_sb2[:], wp_r)
            bqk_sb = [bqk_sb4[:, m : m + 1] for m in range(2 * GW // 128)]
            wp_sb = [wp_sb2[:, j * E : (j + 1) * E] for j in range(2)]

            # ---- warmups: junk matmuls release the PE clock throttle (HAM)
            # while the first DMAs land; dummy exp preloads the ACT table ----
            warm = ppool.tile([128, 512], f32, tag="p", name="warm")
            for w in range(14):
                nc.tensor.matmul(
                    warm[:, 0:320], warm_w[:, 0:128], warm_w[:],
                    start=True, stop=True,
                )
            wexp = cst.tile([128, 1], f32, tag="wexp")
            nc.scalar.activation(wexp[:], wexp_src[:], Exp, scale=0.0)

            # ---- qkT: [2*GW, S] = wqk.T @ x.T ----
            qk_sb = []
            for m in range(2 * GW // 128):
                qk_sb.append(qkp.tile([128, S], bf16, tag="qk", name=f"qk{m}"))

            # ---- V: [S, GW] with ones column per head ----
            v_sb = [None] * NKT

            class VGroup:
                """V tile mt as a chain of e-matmul steps (ppool slot held)."""

                def __init__(self, mt):
                    self.mt = mt
                    self.pv = None
                    self.e = 0

                def step(self, k):
                    if self.pv is None:
                        self.pv = ppool.tile([128, 512], f32, tag="p", name="pv")
                    for _ in range(k):
                        e = self.e
                        nc.tensor.matmul(
                            self.pv[:, 0:GW],
                            xt[e][:, self.mt * 128 : (self.mt + 1) * 128],
                            wv_sb[:, e * GW : (e + 1) * GW],
                            start=(e == 0),
                            stop=(e == ET - 1),
                        )
                        self.e += 1
                    if self.e == ET:
                        vt = vp.tile([128, HPC * VW], bf16, tag="v", name="vt")
                        vt_h = vt.rearrange("p (h w) -> p h w", w=VW)
                        nc.vector.tensor_add(
                            vt_h[:, :, 0:D],
                            self.pv[:, 0:GW].rearrange("p (h d) -> p h d", d=D),
                            bv_bc.rearrange("p (h d) -> p h d", d=D),
                        )
                        nc.vector.memset(vt_h[:, :, D : D + 1], 1.0)
                        v_sb[self.mt] = vt
                        self.pv = None
                        return True
                    return False

            class QktChain:
                """qkT n-tile (m, n) as a chain of e-matmul steps."""

                def __init__(self, m, n):
                    self.m, self.n = m, n
                    self.pq = None
                    self.e = 0

                def step(self, k):
                    if self.pq is None:
                        self.pq = ppool.tile([128, 512], f32, tag="p", name="pq")
                    for _ in range(k):
                        e = self.e
                        nc.tensor.matmul(
                            self.pq[:],
                            wqk_sb[e][:, self.m * 128 : (self.m + 1) * 128],
                            xt[e][:, self.n * 512 : (self.n + 1) * 512],
                            start=(e == 0),
                            stop=(e == ET - 1),
                        )
                        self.e += 1
                    if self.e == ET:
                        nc.vector.tensor_scalar_add(
                            qk_sb[self.m][:, self.n * 512 : (self.n + 1) * 512],
                            self.pq[:],
                            bqk_sb[self.m],
                        )
                        self.pq = None
                        return True
                    return False

            # ---- pre-phase: e-major over (K^T pair0 both halves, Q^T pair0
            # lo half, V tiles 0-1) so each (wqk[e], xt[e]) DMA arrival feeds
            # ~8 matmuls; PSUM: 2 spool psA + 2 chains + 2 V chains is tight
            # but the m0 chains and V chains rotate through 4 ppool slots ----
            psA = [spool.tile([128, 1024], f32, tag="s", name=f"psA{i}") for i in range(2)]
            m0c = [QktChain(0, n) for n in range(2)]
            v01 = [VGroup(mt) for mt in range(2)]
            for e in range(ET):
                for half in range(2):  # K^T pair0: m=2, n-tiles (0,1) and (2,3)
                    for n in (2 * half, 2 * half + 1):
                        nc.tensor.matmul(
                            psA[half][:, (n - 2 * half) * 512 : (n - 2 * half + 1) * 512],
                            wqk_sb[e][:, 2 * 128 : 3 * 128],
                            xt[e][:, n * 512 : (n + 1) * 512],
                            start=(e == 0),
                            stop=(e == ET - 1),
                        )
                m0c[0].step(1)
                m0c[1].step(1)
                if e >= 4:  # wv lands a few tiles in; V consumes xt[0..] then
                    v01[0].step(2)
                    v01[1].step(2)
            for half in range(2):
                nc.vector.tensor_scalar_add(
                    qk_sb[2][:, 2 * half * 512 : (2 * half + 2) * 512],
                    psA[half][:],
                    bqk_sb[2],
                )

            # ---- fill work queues ----
            # u0: V tiles 2..15 at 2/kg. u1-u4: remaining qkT as 2-matmul
            # chunks (group order meets each unit's scores deadlines).
            v_fill = [VGroup(mt) for mt in range(2, NKT)]
            chunk_groups = {
                1: [(0, 2), (0, 3), (3, 0)],
                2: [(3, 1), (3, 2), (3, 3)],
                3: [(1, 0), (1, 1)],
                4: [(1, 2), (1, 3)],
            }
            chunk_quota = {
                1: [2, 1, 2, 1, 2, 1, 2, 1],
                2: [2, 1, 2, 1, 2, 1, 2, 1],
                3: [1, 1, 1, 1, 1, 1, 1, 1],
                4: [1, 1, 1, 1, 1, 1, 1, 1],
            }

            # ---- attention ----
            yn_sb = [ynp.tile([128, S], bf16, tag="yn", name=f"yn{j}") for j in range(2)]

            def emit_proj(mt):
                # both E-halves of out rows mt*128.. in one go: 4 matmuls,
                # 2 copies (fp32->bf16), 1 output DMA with 2KB lines
                ot = op.tile([128, 1024], bf16, tag="o", name="ot")
                for nt in range(2):
                    pp = ppool.tile([128, 512], f32, tag="p", name="pp")
                    for j in range(2):
                        nc.tensor.matmul(
                            pp[:],
                            yn_sb[j][:, mt * 128 : (mt + 1) * 128],
                            wp_sb[j][:, nt * 512 : (nt + 1) * 512],
                            start=(j == 0),
                            stop=(j == 1),
                        )
                    nc.vector.tensor_copy(ot[:, nt * 512 : (nt + 1) * 512], pp[:])
                nc.sync.dma_start(out_t[mt][:], ot[:])

            def emit_av(unit, kg):
                pair, eT, pav, _ = unit
                for sub in range(2):
                    kt = 2 * kg + sub
                    for h in range(2):
                        nc.tensor.matmul(
                            pav[h][0:VW, :],
                            v_sb[kt][:, (2 * pair + h) * VW : (2 * pair + h + 1) * VW],
                            eT[:, kt * 1024 + 512 * h : kt * 1024 + 512 * (h + 1)],
                            start=(kt == 0),
                            stop=(kt == NKT - 1),
                        )

            def emit_norm(unit):
                pair, _, pav, qs = unit
                for h in range(2):
                    rs = rp.tile([1, 512], f32, tag="rs", name="rs")
                    nc.vector.tensor_copy(rs[:], pav[h][D : D + 1, :])
                    r = rp.tile([1, 512], f32, tag="r", name="r")
                    nc.vector.reciprocal_approx_fast(r[:], rs[:])
                    rb = rp.tile([64, 512], f32, tag="rb", name="rb")
                    nc.gpsimd.partition_broadcast(rb[:], r[:])
                    nc.vector.tensor_mul(
                        yn_sb[pair][64 * h : 64 * (h + 1), qs], pav[h][0:D, :], rb[:]
                    )

            # attn@v lags two kg steps behind scores/exp so the PE always has
            # scores(kg+1) first in its stream when a score slot frees
            AV_LAG = 2
            av_queue = []  # (unit, kg); emit_norm(unit) after its kg==last av

            def pop_av():
                u, k = av_queue.pop(0)
                emit_av(u, k)
                if k == NKT // 2 - 1:
                    emit_norm(u)

            for u in range(2 * NQT):  # pair-major: all pair-0 units first
                pair, qt = u // NQT, u % NQT
                qs = slice(qt * 512, (qt + 1) * 512)
                # c_proj of qt-1 (both pairs done) drips into pair-1 units
                proj_work = (
                    list(range((qt - 1) * 4, qt * 4)) if pair == 1 and qt > 0 else []
                )
                qT = qk_sb[pair]          # Q^T of heads (2*pair, 2*pair+1)
                kT = qk_sb[2 + pair]      # K^T of same heads
                # merged exp tile: chunk kt holds [h0 512 | h1 512]
                eT = ep.tile([128, NKT * 1024], bf16, tag="e", name="eT")
                pav = [
                    ppool.tile([128, 512], f32, tag="p", name=f"pav{h}")
                    for h in range(2)
                ]
                unit = (pair, eT, pav, qs)
                chains = [QktChain(m, n) for (m, n) in chunk_groups.get(u, [])]
                chunks = [c for c in chains for _ in range(4)]
                for kg in range(NKT // 2):
                    # slot `sub` holds both heads of kt=2*kg+sub, so one
                    # exp covers a head pair and frees the slot the next
                    # step's first score pair needs
                    sl = [
                        spool.tile([128, 1024], f32, tag="s", name=f"sl{i}")
                        for i in range(2)
                    ]
                    prev_mm = None
                    for sub in range(2):  # kt pair
                        kt = 2 * kg + sub
                        for h in range(2):  # row-packed head pair
                            pr = slice(64 * h, 64 * (h + 1))
                            mm = nc.tensor.matmul(
                                sl[sub][:, h * 512 : (h + 1) * 512],
                                kT[pr, kt * 128 : (kt + 1) * 128],
                                qT[pr, qs],
                                start=True,
                                stop=True,
                            )
                            if prev_mm is not None:
                                # keep the h0/h1 pair adjacent in the PE
                                # stream so the 64-row tiles overlap
                                add_dep_helper(
                                    mm.ins, prev_mm.ins, sync=False,
                                    reason="row-pack order",
                                )
                            prev_mm = mm
                    for sub in range(2):
                        kt = 2 * kg + sub
                        nc.scalar.activation(
                            eT[:, kt * 1024 : (kt + 1) * 1024],
                            sl[sub][:],
                            Exp,
                            scale=1.0 / np.sqrt(D),
                        )
                    av_queue.append((unit, kg))
                    if len(av_queue) > AV_LAG:
                        pop_av()
                    # fill injection
                    if u == 0:
                        for _ in range(2):
                            if v_fill:
                                g = v_fill[0]
                                g.step(ET)
                                if g.e == ET:
                                    v_fill.pop(0)
                    elif chunks:
                        for _ in range(chunk_quota[u][kg]):
                            if not chunks:
                                break
                            chunks.pop(0).step(2)
                    if proj_work and kg >= 1:
                        emit_proj(proj_work.pop(0))
                        if kg >= NKT // 2 - 2 and proj_work:
                            emit_proj(proj_work.pop(0))
                assert not proj_work
            while av_queue:
                pop_av()
            for mt in range((NQT - 1) * 4, NQT * 4):
                emit_proj(mt)

    nc.compile()
    return nc


def _get_nc():
    if "nc" not in _CACHE:
        _CACHE["nc"] = _build()
    return _CACHE["nc"]


def _shard_inputs(x, W_attn, b_attn, W_proj):
    """Per-core input dicts; core c = 4*b + g."""
    in_maps = []
    for c in range(NCORES):
        b, g = divmod(c, 4)
        cs = slice(GW * g, GW * (g + 1))
        xTb = np.ascontiguousarray(x[b].T).astype(BF16)
        wqk = np.concatenate(
            [W_attn[:, cs], W_attn[:, E + GW * g : E + GW * (g + 1)]], axis=1
        ).astype(BF16)
        wv = np.ascontiguousarray(W_attn[:, 2 * E + GW * g : 2 * E + GW * (g + 1)]).astype(BF16)
        bqk = np.concatenate(
            [b_attn[cs], b_attn[E + GW * g : E + GW * (g + 1)]]
        ).astype(np.float32)[:, None]
        bv = np.ascontiguousarray(b_attn[2 * E + GW * g : 2 * E + GW * (g + 1)]).astype(np.float32)
        wpc = np.ascontiguousarray(W_proj[cs, :]).astype(BF16)
        # pre-arrange wv/bqk/wp into the kernel's single-issue 2D layouts
        wv_r = wv.reshape(ET, 128, GW).transpose(1, 0, 2).reshape(128, ET * GW)
        bqk_r = bqk.reshape(2 * GW // 128, 128).T
        wp_r = wpc.reshape(2, 128, E).transpose(1, 0, 2).reshape(128, 2 * E)
        in_maps.append(
            {
                "xT": np.ascontiguousarray(xTb),
                "wqk": np.ascontiguousarray(wqk),
                "wv": np.ascontiguousarray(wv_r),
                "bqk": np.ascontiguousarray(bqk_r),
                "bv": bv,
                "wp": np.ascontiguousarray(wp_r),
            }
        )
    return in_maps


def kernel(x, W_attn, b_attn, W_proj, b_proj, _trace=False):
    from concourse import bass_utils

    x = np.asarray(x, dtype=np.float32)
    W_attn = np.asarray(W_attn, dtype=np.float32)
    b_attn = np.asarray(b_attn, dtype=np.float32)
    W_proj = np.asarray(W_proj, dtype=np.float32)
    b_proj = np.asarray(b_proj, dtype=np.float32)

    nc = _get_nc()
    in_maps = _shard_inputs(x, W_attn, b_attn, W_proj)
    res = bass_utils.run_bass_kernel_spmd(
        nc, in_maps, core_ids=list(range(NCORES)), trace=_trace
    )
    _CACHE["last_result"] = res
    out = np.zeros((B, S, E), dtype=np.float32)
    for c in range(NCORES):
        out[c // 4] += np.asarray(res.results[c]["out"], dtype=np.float32)
    out += b_proj
    return out


# revision 18
# speedup vs baseline: 1.0153x; 1.0153x over previous
"""Multi-head self-attention (B=2, S=2048, E=1024, H=16) on 8 NeuronCores.

Sharding: core c handles batch b = c // 4 and head group g = c % 4 (4 heads).
Each core computes: qkv projection for its heads, attention, and a partial
c_proj (row-slice of W_proj); the host sums the 4 partials per batch.

Device pipeline (all matmuls bf16 inputs, fp32 PSUM accumulation):
  qkT  [2*GW, S]  = (wqk slice).T-stationary @ xT          (Q^T and K^T, head-major)
  V    [S, GW]    = xT-stationary @ wv                     (natural layout, +ones col)
  sT   [kt, q]    = K^T.T @ Q^T  (per head, 64-deep contraction, row-packed pairs)
  expT            = exp(0.125 * sT)  on ACT, bf16 out      (no max-sub: |s/8| <~ 6)
  yT'  [65, q]    = V'.T @ expT  (row 64 = softmax sums via ones column)
  yn   [64, q]    = yT' * (1/sums) broadcast               (per-head normalization)
  out  [S, E]    += yn.T-stationary @ wp                   (partial; host reduces)

Schedule: warmup matmuls run on a memset tile (no DMA dep) right after the
preamble; the pre-phase consumes (wqk[e], xt[e]) pairs e-major as they land
(K^T pair0 + Q^T pair0 lo + first two V tiles); the remaining qkT work is
dripped as 2-matmul chunks through units 1-4 so the PE never starves while
the exp-bound attention windows run; c_proj of qt-1 drips into pair-1 units.
Output partials are written bf16 (host reduce in fp32).
"""

import numpy as np
import ml_dtypes

B, S, E, H, D = 2, 2048, 1024, 16, 64
HPC = 4              # heads per core
GW = HPC * D         # 256: per-core width of q/k/v blocks
VW = D + 1           # 65: v columns + ones column
NCORES = 8
ET = E // 128        # 8 contraction tiles for the projections
NQT = S // 512       # 4 query-column tiles
NKT = S // 128       # 16 key tiles
BF16 = ml_dtypes.bfloat16

_CACHE = {}


def _build():
    import concourse.bass as bass
    import concourse.mybir as mybir
    import concourse.tile as tile
    from concourse.tile import add_dep_helper
    from concourse import bacc

    f32, bf16 = mybir.dt.float32, mybir.dt.bfloat16
    Exp = mybir.ActivationFunctionType.Exp

    nc = bacc.Bacc("TRN2", target_bir_lowering=False, debug=False, num_devices=NCORES)
    xT = nc.dram_tensor("xT", [E, S], bf16, kind="ExternalInput").ap()
    wqk = nc.dram_tensor("wqk", [E, 2 * GW], bf16, kind="ExternalInput").ap()
    # wv/bqk/wp are pre-arranged host-side into single-issue 2D layouts:
    # partition p holds [t0-block | t1-block | ...] for row (t*128+p)
    wv_r = nc.dram_tensor("wv", [128, ET * GW], bf16, kind="ExternalInput").ap()
    bqk_r = nc.dram_tensor("bqk", [128, 2 * GW // 128], f32, kind="ExternalInput").ap()
    bv = nc.dram_tensor("bv", [GW], f32, kind="ExternalInput").ap()
    wp_r = nc.dram_tensor("wp", [128, 2 * E], bf16, kind="ExternalInput").ap()
    out = nc.dram_tensor("out", [S, E], bf16, kind="ExternalOutput").ap()

    xT_t = xT.rearrange("(t p) s -> t p s", p=128)
    wqk_t = wqk.rearrange("(t p) n -> t p n", p=128)
    out_t = out.rearrange("(t p) n -> t p n", p=128)

    with tile.TileContext(nc) as tc:
        with (
            tc.tile_pool(name="xp", bufs=ET) as xp,
            tc.tile_pool(name="wqkp", bufs=ET) as wqkp,
            tc.tile_pool(name="wvp", bufs=1) as wvp,
            tc.tile_pool(name="wpp", bufs=1) as wpp,
            tc.tile_pool(name="cst", bufs=1) as cst,
            tc.tile_pool(name="qkp", bufs=4) as qkp,
            tc.tile_pool(name="vp", bufs=NKT) as vp,
            tc.tile_pool(name="ep", bufs=2) as ep,
            tc.tile_pool(name="ynp", bufs=2) as ynp,
            tc.tile_pool(name="rp", bufs=4) as rp,
            tc.tile_pool(name="op", bufs=4) as op,
            tc.tile_pool(name="spool", bufs=2, space="PSUM") as spool,
            tc.tile_pool(name="ppool", bufs=4, space="PSUM") as ppool,
        ):
            # ---- constants first: warmup stationary + exp-table preload src,
            # so the PE/ACT warmups have no DMA dependency at all ----
            warm_w = cst.tile([128, 320], bf16, tag="warmw")
            nc.vector.memset(warm_w[:], 0.25)
            wexp_src = cst.tile([128, 1], f32, tag="wexpsrc")
            nc.vector.memset(wexp_src[:], 0.0)

            # ---- input DMAs: wqk[e] before xt[e] (pre-phase consumes pairs
            # e-major); small/late-needed tensors batched into single issues ----
            xt, wqk_sb = [], []
            wv_sb = wvp.tile([128, ET * GW], bf16, tag="wv")
            bqk_sb4 = cst.tile([128, 2 * GW // 128], f32, tag="bqk")
            wp_sb2 = wpp.tile([128, 2 * E], bf16, tag="wp")
            bv_bc = cst.tile([128, GW], f32, tag="bv")
            bv_b = bass.AP(tensor=bv.tensor, offset=bv.offset, ap=[[0, 128], *bv.ap])
            for e in range(ET):
                t = wqkp.tile([128, 2 * GW], bf16, tag="wqk", name=f"wqk{e}")
                nc.sync.dma_start(t[:], wqk_t[e])
                wqk_sb.append(t)
                t = xp.tile([128, S], bf16, tag="xt", name=f"xt{e}")
                nc.sync.dma_start(t[:], xT_t[e])
                xt.append(t)
                if e == 1:
                    nc.sync.dma_start(wv_sb[:], wv_r)
                    nc.sync.dma_start(bv_bc[:], bv_b)
                if e == 3:
                    nc.sync.dma_start(bqk_sb4[:], bqk_r)
            nc.sync.dma_start(wp_sb2[:], wp_r)
            bqk_sb = [bqk_sb4[:, m : m + 1] for m in range(2 * GW // 128)]
            wp_sb = [wp_sb2[:, j * E : (j + 1) * E] for j in range(2)]

            # ---- warmups: junk matmuls release the PE clock throttle (HAM)
            # while the first DMAs land; dummy exp preloads the ACT table ----
            warm = ppool.tile([128, 512], f32, tag="p", name="warm")
            for w in range(14):
                nc.tensor.matmul(
                    warm[:, 0:320], warm_w[:, 0:128], warm_w[:],
                    start=True, stop=True,
                )
            wexp = cst.tile([128, 1], f32, tag="wexp")
            nc.scalar.activation(wexp[:], wexp_src[:], Exp, scale=0.0)

            # ---- qkT: [2*GW, S] = wqk.T @ x.T ----
            qk_sb = []
            for m in range(2 * GW // 128):
                qk_sb.append(qkp.tile([128, S], bf16, tag="qk", name=f"qk{m}"))

            # ---- V: [S, GW] with ones column per head ----
            v_sb = [None] * NKT

            class VGroup:
                """V tile mt as a chain of e-matmul steps (ppool slot held)."""

                def __init__(self, mt):
                    self.mt = mt
                    self.pv = None
                    self.e = 0

                def step(self, k):
                    if self.pv is None:
                        self.pv = ppool.tile([128, 512], f32, tag="p", name="pv")
                    for _ in range(k):
                        e = self.e
                        nc.tensor.matmul(
                            self.pv[:, 0:GW],
                            xt[e][:, self.mt * 128 : (self.mt + 1) * 128],
                            wv_sb[:, e * GW : (e + 1) * GW],
                            start=(e == 0),
                            stop=(e == ET - 1),
                        )
                        self.e += 1
                    if self.e == ET:
                        vt = vp.tile([128, HPC * VW], bf16, tag="v", name="vt")
                        vt_h = vt.rearrange("p (h w) -> p h w", w=VW)
                        nc.vector.tensor_add(
                            vt_h[:, :, 0:D],
                            self.pv[:, 0:GW].rearrange("p (h d) -> p h d", d=D),
                            bv_bc.rearrange("p (h d) -> p h d", d=D),
                        )
                        nc.vector.memset(vt_h[:, :, D : D + 1], 1.0)
                        v_sb[self.mt] = vt
                        self.pv = None
                        return True
                    return False

            class QktChain:
                """qkT n-tile (m, n) as a chain of e-matmul steps."""

                def __init__(self, m, n):
                    self.m, self.n = m, n
                    self.pq = None
                    self.e = 0

                def step(self, k):
                    if self.pq is None:
                        self.pq = ppool.tile([128, 512], f32, tag="p", name="pq")
                    for _ in range(k):
                        e = self.e
                        nc.tensor.matmul(
                            self.pq[:],
                            wqk_sb[e][:, self.m * 128 : (self.m + 1) * 128],
                            xt[e][:, self.n * 512 : (self.n + 1) * 512],
                            start=(e == 0),
                            stop=(e == ET - 1),
                        )
                        self.e += 1
                    if self.e == ET:
                        nc.vector.tensor_scalar_add(
                            qk_sb[self.m][:, self.n * 512 : (self.n + 1) * 512],
                            self.pq[:],
                            bqk_sb[self.m],
                        )
                        self.pq = None
                        return True
                    return False

            # ---- pre-phase: e-major over (K^T pair0 both halves, Q^T pair0
            # lo half, V tiles 0-1) so each (wqk[e], xt[e]) DMA arrival feeds
            # ~8 matmuls; PSUM: 2 spool psA + 2 chains + 2 V chains is tight
            # but the m0 chains and V chains rotate through 4 ppool slots ----
            psA = [spool.tile([128, 1024], f32, tag="s", name=f"psA{i}") for i in range(2)]
            m0c = [QktChain(0, n) for n in range(2)]
            v01 = [VGroup(mt) for mt in range(2)]
            for e in range(ET):
                for half in range(2):  # K^T pair0: m=2, n-tiles (0,1) and (2,3)
                    for n in (2 * half, 2 * half + 1):
                        nc.tensor.matmul(
                            psA[half][:, (n - 2 * half) * 512 : (n - 2 * half + 1) * 512],
                            wqk_sb[e][:, 2 * 128 : 3 * 128],
                            xt[e][:, n * 512 : (n + 1) * 512],
                            start=(e == 0),
                            stop=(e == ET - 1),
                        )
                m0c[0].step(1)
                m0c[1].step(1)
                if e >= 2:  # wv lands a few tiles in; V consumes xt[0..] then
                    k = 1 if e < 6 else 2
                    v01[0].step(k)
                    v01[1].step(k)
            for half in range(2):
                nc.vector.tensor_scalar_add(
                    qk_sb[2][:, 2 * half * 512 : (2 * half + 2) * 512],
                    psA[half][:],
                    bqk_sb[2],
                )

            # ---- fill work queues ----
            # u0: V tiles 2..15 at 2/kg. u1-u4: remaining qkT as 2-matmul
            # chunks (group order meets each unit's scores deadlines).
            v_fill = [VGroup(mt) for mt in range(2, NKT)]
            chunk_groups = {
                1: [(0, 2), (0, 3), (3, 0)],
                2: [(3, 1), (3, 2)],
                3: [(3, 3), (1, 0), (1, 1)],
                4: [(1, 2), (1, 3)],
            }
            chunk_quota = {
                1: [2, 2, 1, 2, 1, 2, 1, 1],
                2: [1, 1, 1, 1, 1, 1, 1, 1],
                3: [2, 2, 1, 2, 1, 2, 1, 1],
                4: [1, 1, 1, 1, 1, 1, 1, 1],
            }
            # unit schedule: (pair, q_start, q_width)
            unit_specs = (
                [(0, qt * 512, 512) for qt in range(NQT)]
                + [(1, qt * 512, 512) for qt in range(NQT)]
            )
            # c_proj drip: unit idx -> (list of mt, list of kg slots)
            proj_sched = {
                5: (list(range(0, 4)), [2, 3, 5, 7]),
                6: (list(range(4, 8)), [2, 3, 5, 7]),
                7: (list(range(8, 12)), [2, 3, 5, 7]),
            }
            proj_tail = [12, 13, 14, 15]

            # ---- attention ----
            yn_sb = [ynp.tile([128, S], bf16, tag="yn", name=f"yn{j}") for j in range(2)]

            def emit_proj(mt):
                # both E-halves of out rows mt*128.. in one go: 4 matmuls,
                # 2 copies (fp32->bf16), 1 output DMA with 2KB lines
                ot = op.tile([128, 1024], bf16, tag="o", name="ot")
                for nt in range(2):
                    pp = ppool.tile([128, 512], f32, tag="p", name="pp")
                    for j in range(2):
                        nc.tensor.matmul(
                            pp[:],
                            yn_sb[j][:, mt * 128 : (mt + 1) * 128],
                            wp_sb[j][:, nt * 512 : (nt + 1) * 512],
                            start=(j == 0),
                            stop=(j == 1),
                        )
                    nc.vector.tensor_copy(ot[:, nt * 512 : (nt + 1) * 512], pp[:])
                nc.sync.dma_start(out_t[mt][:], ot[:])

            def emit_av(unit, kg):
                pair, eT, pav, _, W = unit
                for sub in range(2):
                    kt = 2 * kg + sub
                    for h in range(2):
                        nc.tensor.matmul(
                            pav[h][0:VW, 0:W],
                            v_sb[kt][:, (2 * pair + h) * VW : (2 * pair + h + 1) * VW],
                            eT[:, kt * 2 * W + W * h : kt * 2 * W + W * (h + 1)],
                            start=(kt == 0),
                            stop=(kt == NKT - 1),
                        )

            def emit_norm(unit):
                pair, _, pav, qs, W = unit
                for h in range(2):
                    rs = rp.tile([1, W], f32, tag="rs", name="rs")
                    nc.vector.tensor_copy(rs[:], pav[h][D : D + 1, 0:W])
                    r = rp.tile([1, W], f32, tag="r", name="r")
                    nc.vector.reciprocal_approx_fast(r[:], rs[:])
                    rb = rp.tile([64, W], f32, tag="rb", name="rb")
                    nc.gpsimd.partition_broadcast(rb[:], r[:])
                    nc.vector.tensor_mul(
                        yn_sb[pair][64 * h : 64 * (h + 1), qs], pav[h][0:D, 0:W], rb[:]
                    )

            # attn@v lags two kg steps behind scores/exp so the PE always has
            # scores(kg+1) first in its stream when a score slot frees
            AV_LAG = 2
            av_queue = []  # (unit, kg); emit_norm(unit) after its kg==last av

            def pop_av():
                u, k = av_queue.pop(0)
                emit_av(u, k)
                if k == NKT // 2 - 1:
                    emit_norm(u)

            for u, (pair, q0, W) in enumerate(unit_specs):
                qs = slice(q0, q0 + W)
                proj_mts, proj_kgs = proj_sched.get(u, ([], []))
                proj_mts, proj_kgs = list(proj_mts), list(proj_kgs)
                qT = qk_sb[pair]          # Q^T of heads (2*pair, 2*pair+1)
                kT = qk_sb[2 + pair]      # K^T of same heads
                # merged exp tile: chunk kt holds [h0 W | h1 W]
                eT = ep.tile([128, NKT * 2 * W], bf16, tag="e", name="eT")
                pav = [
                    ppool.tile([128, 512], f32, tag="p", name=f"pav{h}")
                    for h in range(2)
                ]
                unit = (pair, eT, pav, qs, W)
                chains = [QktChain(m, n) for (m, n) in chunk_groups.get(u, [])]
                chunks = [c for c in chains for _ in range(4)]
                for kg in range(NKT // 2):
                    # slot `sub` holds both heads of kt=2*kg+sub, so one
                    # exp covers a head pair and frees the slot the next
                    # step's first score pair needs
                    sl = [
                        spool.tile([128, 2 * W], f32, tag="s", name=f"sl{i}")
                        for i in range(2)
                    ]
                    prev_mm = None
                    for sub in range(2):  # kt pair
                        kt = 2 * kg + sub
                        for h in range(2):  # row-packed head pair
                            pr = slice(64 * h, 64 * (h + 1))
                            mm = nc.tensor.matmul(
                                sl[sub][:, h * W : (h + 1) * W],
                                kT[pr, kt * 128 : (kt + 1) * 128],
                                qT[pr, qs],
                                start=True,
                                stop=True,
                            )
                            if prev_mm is not None:
                                # keep the h0/h1 pair adjacent in the PE
                                # stream so the 64-row tiles overlap
                                add_dep_helper(
                                    mm.ins, prev_mm.ins, sync=False,
                                    reason="row-pack order",
                                )
                            prev_mm = mm
                    for sub in range(2):
                        kt = 2 * kg + sub
                        nc.scalar.activation(
                            eT[:, kt * 2 * W : (kt + 1) * 2 * W],
                            sl[sub][:],
                            Exp,
                            scale=1.0 / np.sqrt(D),
                        )
                    av_queue.append((unit, kg))
                    if len(av_queue) > AV_LAG:
                        pop_av()
                    # fill injection
                    if u == 0:
                        for _ in range(2):
                            if v_fill:
                                g = v_fill[0]
                                g.step(ET)
                                if g.e == ET:
                                    v_fill.pop(0)
                    elif chunks:
                        for _ in range(chunk_quota[u][kg]):
                            if not chunks:
                                break
                            chunks.pop(0).step(2)
                    if proj_mts and proj_kgs and kg == proj_kgs[0]:
                        proj_kgs.pop(0)
                        emit_proj(proj_mts.pop(0))
                assert not proj_mts
            while av_queue:
                pop_av()
            for mt in proj_tail:
                emit_proj(mt)

    nc.compile()
    return nc


def _get_nc():
    if "nc" not in _CACHE:
        _CACHE["nc"] = _build()
    return _CACHE["nc"]


def _shard_inputs(x, W_attn, b_attn, W_proj):
    """Per-core input dicts; core c = 4*b + g."""
    in_maps = []
    for c in range(NCORES):
        b, g = divmod(c, 4)
        cs = slice(GW * g, GW * (g + 1))
        xTb = np.ascontiguousarray(x[b].T).astype(BF16)
        wqk = np.concatenate(
            [W_attn[:, cs], W_attn[:, E + GW * g : E + GW * (g + 1)]], axis=1
        ).astype(BF16)
        wv = np.ascontiguousarray(W_attn[:, 2 * E + GW * g : 2 * E + GW * (g + 1)]).astype(BF16)
        bqk = np.concatenate(
            [b_attn[cs], b_attn[E + GW * g : E + GW * (g + 1)]]
        ).astype(np.float32)[:, None]
        bv = np.ascontiguousarray(b_attn[2 * E + GW * g : 2 * E + GW * (g + 1)]).astype(np.float32)
        wpc = np.ascontiguousarray(W_proj[cs, :]).astype(BF16)
        # pre-arrange wv/bqk/wp into the kernel's single-issue 2D layouts
        wv_r = wv.reshape(ET, 128, GW).transpose(1, 0, 2).reshape(128, ET * GW)
        bqk_r = bqk.reshape(2 * GW // 128, 128).T
        wp_r = wpc.reshape(2, 128, E).transpose(1, 0, 2).reshape(128, 2 * E)
        in_maps.append(
            {
                "xT": np.ascontiguousarray(xTb),
                "wqk": np.ascontiguousarray(wqk),
                "wv": np.ascontiguousarray(wv_r),
                "bqk": np.ascontiguousarray(bqk_r),
                "bv": bv,
                "wp": np.ascontiguousarray(wp_r),
            }
        )
    return in_maps


def kernel(x, W_attn, b_attn, W_proj, b_proj, _trace=False):
    from concourse import bass_utils

    x = np.asarray(x, dtype=np.float32)
    W_attn = np.asarray(W_attn, dtype=np.float32)
    b_attn = np.asarray(b_attn, dtype=np.float32)
    W_proj = np.asarray(W_proj, dtype=np.float32)
    b_proj = np.asarray(b_proj, dtype=np.float32)

    nc = _get_nc()
    in_maps = _shard_inputs(x, W_attn, b_attn, W_proj)
    res = bass_utils.run_bass_kernel_spmd(
        nc, in_maps, core_ids=list(range(NCORES)), trace=_trace
    )
    _CACHE["last_result"] = res
    out = np.zeros((B, S, E), dtype=np.float32)
    for c in range(NCORES):
        out[c // 4] += np.asarray(res.results[c]["out"], dtype=np.float32)
    out += b_proj
    return out
